# revision 32
# baseline (speedup 1.0000x reference)
"""Trainium2 Bass kernel: HMM forward algorithm (log-space) for AugmentedModel.log_prob.

Probability-domain recurrence with fp8 DoubleRow matmuls:
    w   = alpha ⊙ q_t                  (q_t pre-scaled per (t,b) so mass ~ 1)
    w8  = fp8_e5m2(w ⊙ mask_a)         (per-action one-hot masks, disjoint)
    u   = Σ_a w8_a @ P8[a]             (P8 = fp8_e4m3(512·exp(log_trans)))
    s_t = Σ_s u; alpha ∝ u / s_{t-1}   (lag-1 rescale keeps mass ≈ 512·e^δ)
log_prob[b] = Σ_t ln s_t + Σ_t C_tb − 129·ln 512   (host-side; C = per-(t,b)
pre-scale constants injected via an extra one-hot vocab row).

Matmuls use MatmulPerfMode.DoubleRow: two 128-row K-tiles per instruction at
0.5 cycles/row — 16 matmuls/step instead of 32 bf16 ones at 1 cycle/row, a 4x
cut in PE streaming time.  All 8 action matmuls accumulate into one PSUM tile
(masks are disjoint one-hots, so the sum IS the per-batch selection).

Sharding: data-parallel over batch B=128 -> 16 episodes per core, tables
replicated; no collectives (each core's recurrence is independent).
"""

import numpy as np
from contextlib import ExitStack

T, B, S, A, NO, NR = 128, 128, 512, 8, 64, 16
TT = T + 1
NCORES = 8
BC = B // NCORES          # 16 episodes per core
KC = 4                    # 512 states = 4 chunks of 128 partitions
NCOL = 6                  # one-hot matmul N-chunks: 2064 = 6*344
NCHUNK = (TT * BC) // NCOL
VOC = NO + NR + 2 + A     # 90 one-hot rows
VOCC = VOC + 1            # +1 row carrying the per-(t,b) scale constant
SP = S + 8                # ptab padded: col 512 = R/4 (quantized row sums)
LN512 = float(np.log(512.0))


def _host_prep(regime, seq_o, seq_r, seq_d, seq_a,
               log_emit_o, log_emit_r, log_emit_d, log_policy):
    """Index preprocessing: one-hots, action masks, and per-(t,b) scale C."""
    d_all = np.concatenate([seq_d, np.ones((1, B), np.int32)], 0)        # [TT,B]
    d_cum = np.maximum.accumulate(d_all, 0)                              # [TT,B]
    was_d = np.concatenate([np.zeros((1, B), np.int32), d_cum[:-1]], 0)  # [TT,B]
    a_full = np.concatenate([seq_a, np.zeros((1, B), np.int32)], 0)      # [TT,B]

    # emission+policy log rows (host, f32) -> per-(t,b) scale constants
    lq = (log_emit_o[seq_o] + log_emit_r[seq_r]
          + log_emit_d[d_cum]).astype(np.float32)                        # [TT,B,S]
    lq[was_d == 1] = 0.0
    lq_a = log_policy[a_full].astype(np.float32)
    lq_a[d_cum == 1] = 0.0
    lq_a[:, regime == 1, :] = 0.0
    lq += lq_a
    mx = lq.max(axis=2)
    C = (mx + np.log(np.exp(lq - mx[:, :, None]).mean(axis=2))).astype(np.float32)

    oh = np.zeros((TT, B, VOCC), np.float32)
    tt, bb = np.meshgrid(np.arange(TT), np.arange(B), indexing="ij")
    live = was_d == 0
    oh[tt[live], bb[live], seq_o[live]] = 1.0
    oh[tt[live], bb[live], NO + seq_r[live]] = 1.0
    oh[tt[live], bb[live], NO + NR + d_cum[live]] = 1.0
    act = (d_cum == 0) & (regime[None, :] == 0)
    oh[tt[act], bb[act], NO + NR + 2 + a_full[act]] = 1.0
    oh[:, :, VOC] = -C                                                   # scale row

    msk = (a_full[:, None, :] == np.arange(A)[None, :, None]).astype(np.float32)
    return oh, msk, C


def _bc_insert(ap, axis, count):
    """Insert a 0-stride (broadcast) dim of size `count` at position `axis`."""
    import concourse.bass as bass
    lst = [list(d) for d in ap.ap]
    lst.insert(axis, [0, count])
    return bass.AP(ap.tensor, ap.offset, lst)


def _build_nc(nsteps=TT, nreps=1):
    import concourse.bass as bass  # noqa: F401
    import concourse.bacc as bacc
    import concourse.mybir as mybir
    import concourse.tile as tile

    f32 = mybir.dt.float32
    f32r = mybir.dt.float32r
    bf16 = mybir.dt.bfloat16
    e4 = mybir.dt.float8e4
    e5 = mybir.dt.float8e5
    EXP = mybir.ActivationFunctionType.Exp
    CPY = mybir.ActivationFunctionType.Copy
    MUL = mybir.AluOpType.mult
    ADD = mybir.AluOpType.add
    DR = mybir.MatmulPerfMode.DoubleRow
    AX = mybir.AxisListType.X

    nc = bacc.Bacc(None, target_bir_lowering=False)

    oh_d = nc.dram_tensor("oh", [VOCC, TT * BC], f32r, kind="ExternalInput")
    tbl_d = nc.dram_tensor("tbl", [VOCC, S], f32r, kind="ExternalInput")
    pt_d = nc.dram_tensor("ptab", [128, A, KC, SP], e4, kind="ExternalInput")
    li_d = nc.dram_tensor("linit", [128, KC], f32, kind="ExternalInput")
    mk_d = nc.dram_tensor("msk", [128, TT, A, BC], bf16, kind="ExternalInput")
    id_d = nc.dram_tensor("ident", [BC, BC], bf16, kind="ExternalInput")
    out_d = nc.dram_tensor("out", [BC, TT], f32, kind="ExternalOutput")

    LOOK = 3                  # qm lookahead (steps)

    with tile.TileContext(nc) as tc, ExitStack() as ctx:
        const = ctx.enter_context(tc.tile_pool(name="const", bufs=1))
        qmpool = ctx.enter_context(tc.tile_pool(name="qm", bufs=LOOK + 3))
        w8pool = ctx.enter_context(tc.tile_pool(name="w8", bufs=2))
        spool = ctx.enter_context(tc.tile_pool(name="ssum", bufs=2))
        nrm = ctx.enter_context(tc.tile_pool(name="nrm", bufs=3))
        pp = ctx.enter_context(tc.tile_pool(name="ppsum", bufs=2, space="PSUM"))
        ptp = ctx.enter_context(tc.tile_pool(name="tpsum", bufs=1, space="PSUM"))
        lqp = pp

        ptab = const.tile([128, A, KC, SP], e4)
        qbuf = const.tile([128, KC, TT * BC], bf16)
        tbl = const.tile([VOCC, S], f32r)
        oh = const.tile([VOCC, TT * BC], f32r)
        ident = const.tile([BC, BC], bf16)
        alpha0 = const.tile([128, KC, 1], f32)
        li = const.tile([128, KC], f32)
        ellbuf = const.tile([BC, TT], f32)
        mall = const.tile([128, TT, A, BC], bf16)

        nc.sync.dma_start(tbl[:], tbl_d[:])
        nc.sync.dma_start(oh[:], oh_d[:])
        nc.sync.dma_start(ident[:], id_d[:])
        nc.sync.dma_start(li[:], li_d[:])
        nc.sync.dma_start(ptab[:], pt_d[:])
        # masks resident in SBUF: chunked so early steps land first
        for ch in range(4):
            t0c = (TT * ch) // 4
            t1c = (TT * (ch + 1)) // 4
            nc.sync.dma_start(mall[:, t0c:t1c], mk_d[:, t0c:t1c])
        nc.scalar.activation(alpha0[:, :, 0], li[:], EXP)

        # emission rows: lq = tbl.T @ onehot (+C row), then q = exp(lq) -> bf16
        # j-outer so early time steps are ready first
        for j in range(NCOL):
            for mc in range(KC):
                lq = lqp.tile([128, NCHUNK], f32, tag="lq")
                nc.tensor.matmul(
                    lq[:],
                    tbl[:, mc * 128:(mc + 1) * 128],
                    oh[:, j * NCHUNK:(j + 1) * NCHUNK],
                    start=True, stop=True,
                )
                nc.scalar.activation(
                    qbuf[:, mc, j * NCHUNK:(j + 1) * NCHUNK], lq[:], EXP
                )

        def emit_qm(t):
            """qm[p,a,kc,b] = q_t[p,kc,b] * mask_t[p,a,b]  (bf16, off-chain)"""
            qm = qmpool.tile([128, A, KC, BC], bf16, tag="qm")
            qs = qbuf[:, :, t * BC:(t + 1) * BC]
            nc.vector.tensor_tensor(
                qm[:], _bc_insert(qs, 1, A), _bc_insert(mall[:, t], 2, KC), MUL
            )
            return qm

        def emit_ell(t, recip_prev, uh1_, want_recip=True):
            """s_t = (w8·R column) * recip_{t-1}; then recip_t = 1/s_t."""
            if recip_prev is None:
                nc.vector.tensor_copy(ellbuf[:, t:t + 1], uh1_[:, 256:257])
            else:
                nc.vector.tensor_tensor(ellbuf[:, t:t + 1], uh1_[:, 256:257],
                                        recip_prev[:], MUL)
            if not want_recip:
                return None
            rec = nrm.tile([BC, 1], f32, tag="rec")
            nc.vector.reciprocal(rec[:], ellbuf[:, t:t + 1])
            return rec

        for _rep in range(nreps):
         recip = None
         prev_ell = None          # (t, recip_prev, uh1) pending measurement
         qms = {t: emit_qm(t) for t in range(LOOK)}
         uT_prev = (None, None)
         for t in range(nsteps):
            qm = qms.pop(t)

            # w8[p,a,kc,b] = fp8e5(alpha[p,kc,b] * qm[p,a,kc,b]).  k-pair j0
            # is split into action-halves so the first matmul of the step
            # waits on a 258ns op, not a 392ns one.
            w8 = w8pool.tile([128, A, KC, BC], e5, tag="w8")
            def _w8src(ks, na):
                if t == 0 and _rep == 0:
                    return _bc_insert(_bc_insert(alpha0[:, ks, 0], 1, na), 3, BC)
                j = ks.start // 2
                return _bc_insert(uT_prev[j][:, :, :], 1, na)
            k0, k1 = slice(0, 2), slice(2, 4)
            for asl in (slice(0, 4), slice(4, 8)):
                nc.vector.tensor_tensor(w8[:, asl, k0, :], _w8src(k0, 4),
                                        qm[:, asl, k0, :], MUL)
            nc.vector.tensor_tensor(w8[:, :, k1, :], _w8src(k1, A),
                                    qm[:, :, k1, :], MUL)

            # previous step's mass + reciprocal: emitted here (after the w8
            # muls) so they don't head-of-line-block the DVE sequencer
            # between c0(t-1) and this step's w8 muls.
            if prev_ell is not None:
                recip = emit_ell(*prev_ell)
            if t + LOOK < nsteps:
                qms[t + LOOK] = emit_qm(t + LOOK)

            # u = Σ_a w8_a @ P8[a] : DoubleRow matmuls split into N-halves
            # (separate PSUM tiles).  Phase order balances the two critical
            # cycles: j0L, j0R(a0-3), j1L, j0R(a4-7), j1R — so uh0 stops at
            # ~60% of the stream and the j1 operands (which trail the
            # previous step's second copy) are consumed late.  uh1 carries 4
            # extra columns: col 256 accumulates w8 · R (R = quantized-P8
            # row sums / 4) — the raw step mass lands in PSUM for free.
            uh0 = pp.tile([BC, S // 2], f32, tag="u0")
            uh1 = pp.tile([BC, S // 2 + 4], f32, tag="u1")
            cl, cr = slice(0, 256), slice(256, 516)
            phases = [(0, uh0, cl, range(A)), (0, uh1, cr, range(5)),
                      (1, uh0, cl, range(A)), (0, uh1, cr, range(5, A)),
                      (1, uh1, cr, range(A))]
            for j, u_, cols, arng in phases:
                for a in arng:
                    nc.tensor.matmul(
                        u_[:],
                        w8[:, a, 2 * j:2 * j + 2, :],
                        ptab[:, a, 2 * j:2 * j + 2, cols],
                        start=(j == 0 and a == 0),
                        stop=(j == 1 and a == A - 1),
                        perf_mode=DR,
                        skip_group_check=True,
                    )

            # c half 0 on DVE (lower output latency -> transposes start
            # sooner), half 1 on ACT (parallel engine); transpose each half
            # into its own uT tile.
            c = spool.tile([BC, S], bf16, tag="c")
            sc = recip[:] if recip is not None else 1.0
            last = t == nsteps - 1 and _rep == nreps - 1
            uTs = [None, None]
            for h, u_ in enumerate([uh0, uh1]):
                cols = slice(h * (S // 2), (h + 1) * (S // 2))
                if h == 0:
                    nc.vector.tensor_scalar(c[:, cols], u_[:, 0:256], sc,
                                            None, MUL)
                else:
                    nc.scalar.activation(c[:, cols], u_[:, 0:256], CPY,
                                         scale=sc)
                if not last:
                    uTh = ptp.tile([128, 2, BC], bf16, tag=f"uT{h}",
                                   name=f"uT{h}")
                    for k in range(2):
                        kc = 2 * h + k
                        nc.tensor.matmul(
                            uTh[:, k, :], c[:, kc * 128:(kc + 1) * 128],
                            ident[:], is_transpose=True,
                            start=True, stop=True,
                        )
                    uTs[h] = uTh
            uT_prev = tuple(uTs)

            prev_ell = (t, recip, uh1)

         # flush the final step's mass measurement
         emit_ell(prev_ell[0], prev_ell[1], prev_ell[2],
                  want_recip=_rep < nreps - 1)

        nc.sync.dma_start(out_d[:], ellbuf[:])

    nc.compile()
    return nc


_NC = None


def _get_nc():
    global _NC
    if _NC is None:
        _NC = _build_nc()
    return _NC


def make_in_maps(regime, seq_o, seq_r, seq_d, seq_a,
                 log_init, log_trans, log_emit_o, log_emit_r, log_emit_d,
                 log_policy):
    import ml_dtypes

    oh, msk, C = _host_prep(
        np.asarray(regime), np.asarray(seq_o), np.asarray(seq_r),
        np.asarray(seq_d), np.asarray(seq_a),
        np.asarray(log_emit_o, np.float32), np.asarray(log_emit_r, np.float32),
        np.asarray(log_emit_d, np.float32), np.asarray(log_policy, np.float32),
    )
    tbl = np.concatenate(
        [log_emit_o, log_emit_r, log_emit_d, log_policy,
         np.ones((1, S), np.float32)], 0
    ).astype(np.float32)                                         # [91, 512]
    P8 = (512.0 * np.exp(np.asarray(log_trans, np.float64))).astype(np.float32)
    P8q = np.ascontiguousarray(
        P8.reshape(A, KC, 128, S).transpose(2, 0, 1, 3)
    ).astype(ml_dtypes.float8_e4m3)                              # [128,A,KC,S]
    ptab = np.zeros((128, A, KC, SP), ml_dtypes.float8_e4m3)
    ptab[:, :, :, :S] = P8q
    # col S: R/4 where R = row sums of the quantized P8 (mass measurement)
    ptab[:, :, :, S] = (P8q.astype(np.float32).sum(-1) / 4.0
                        ).astype(ml_dtypes.float8_e4m3)
    linit = np.ascontiguousarray(np.asarray(log_init, np.float32).reshape(KC, 128).T)
    ident = np.eye(BC, dtype=ml_dtypes.bfloat16)

    in_maps = []
    for c in range(NCORES):
        bs = c * BC
        ohc = np.ascontiguousarray(
            oh[:, bs:bs + BC, :].transpose(2, 0, 1).reshape(VOCC, TT * BC)
        )
        mskc = np.ascontiguousarray(
            np.broadcast_to(msk[None, :, :, bs:bs + BC], (128, TT, A, BC))
        ).astype(ml_dtypes.bfloat16)
        in_maps.append({
            "oh": ohc, "tbl": tbl, "ptab": ptab, "linit": linit,
            "msk": mskc, "ident": ident,
        })
    return in_maps, C


def kernel(regime, seq_o, seq_r, seq_d, seq_a,
           log_init, log_trans, log_emit_o, log_emit_r, log_emit_d,
           log_policy, _trace=False):
    from concourse.bass_utils import run_bass_kernel_spmd

    nc = _get_nc()
    in_maps, C = make_in_maps(
        regime, seq_o, seq_r, seq_d, seq_a, log_init, log_trans,
        log_emit_o, log_emit_r, log_emit_d, log_policy,
    )
    res = run_bass_kernel_spmd(nc, in_maps, core_ids=list(range(NCORES)),
                               trace=_trace)
    ell = np.concatenate([r["out"].reshape(BC, TT) for r in res.results])  # [B,TT]
    logp = (np.log(ell.astype(np.float64)).sum(1)
            + C.astype(np.float64).sum(0) - TT * LN512 + np.log(4.0))
    if _trace:
        kernel._last_results = res
    return logp.astype(np.float32)


# revision 44
# speedup vs baseline: 4343.2152x; 4343.2152x over previous
"""Trainium2 Bass kernel: HMM forward algorithm (log-space) for AugmentedModel.log_prob.

Probability-domain recurrence with fp8 DoubleRow matmuls:
    w   = alpha ⊙ q_t                  (q_t pre-scaled per (t,b) so mass ~ 1)
    w8  = fp8_e5m2(w ⊙ mask_a)         (per-action one-hot masks, disjoint)
    u   = Σ_a w8_a @ P8[a]             (P8 = fp8_e4m3(512·exp(log_trans)))
    s_t = Σ_s u; alpha ∝ u / s_{t-1}   (lag-1 rescale keeps mass ≈ 512·e^δ)
log_prob[b] = Σ_t ln s_t + Σ_t C_tb − 129·ln 512   (host-side; C = per-(t,b)
pre-scale constants injected via an extra one-hot vocab row).

Matmuls use MatmulPerfMode.DoubleRow: two 128-row K-tiles per instruction at
0.5 cycles/row — 16 matmuls/step instead of 32 bf16 ones at 1 cycle/row, a 4x
cut in PE streaming time.  All 8 action matmuls accumulate into one PSUM tile
(masks are disjoint one-hots, so the sum IS the per-batch selection).

Sharding: data-parallel over batch B=128 -> 16 episodes per core, tables
replicated; no collectives (each core's recurrence is independent).
"""

import numpy as np
from contextlib import ExitStack

T, B, S, A, NO, NR = 128, 128, 512, 8, 64, 16
TT = T + 1
NCORES = 8
BC = B // NCORES          # 16 episodes per core
KC = 4                    # 512 states = 4 chunks of 128 partitions
NCOL = 6                  # one-hot matmul N-chunks: 2064 = 6*344
NCHUNK = (TT * BC) // NCOL
VOC = NO + NR + 2 + A     # 90 one-hot rows
VOCC = VOC + 1            # +1 row carrying the per-(t,b) scale constant
SP = S + 8                # ptab padded: col 512 = R/4 (quantized row sums)
LN512 = float(np.log(512.0))


def _host_prep(regime, seq_o, seq_r, seq_d, seq_a,
               log_emit_o, log_emit_r, log_emit_d, log_policy):
    """Index preprocessing: one-hots, action masks, and per-(t,b) scale C."""
    d_all = np.concatenate([seq_d, np.ones((1, B), np.int32)], 0)        # [TT,B]
    d_cum = np.maximum.accumulate(d_all, 0)                              # [TT,B]
    was_d = np.concatenate([np.zeros((1, B), np.int32), d_cum[:-1]], 0)  # [TT,B]
    a_full = np.concatenate([seq_a, np.zeros((1, B), np.int32)], 0)      # [TT,B]

    # emission+policy log rows (host, f32) -> per-(t,b) scale constants
    lq = (log_emit_o[seq_o] + log_emit_r[seq_r]
          + log_emit_d[d_cum]).astype(np.float32)                        # [TT,B,S]
    lq[was_d == 1] = 0.0
    lq_a = log_policy[a_full].astype(np.float32)
    lq_a[d_cum == 1] = 0.0
    lq_a[:, regime == 1, :] = 0.0
    lq += lq_a
    mx = lq.max(axis=2)
    C = (mx + np.log(np.exp(lq - mx[:, :, None]).mean(axis=2))).astype(np.float32)

    oh = np.zeros((TT, B, VOCC), np.float32)
    tt, bb = np.meshgrid(np.arange(TT), np.arange(B), indexing="ij")
    live = was_d == 0
    oh[tt[live], bb[live], seq_o[live]] = 1.0
    oh[tt[live], bb[live], NO + seq_r[live]] = 1.0
    oh[tt[live], bb[live], NO + NR + d_cum[live]] = 1.0
    act = (d_cum == 0) & (regime[None, :] == 0)
    oh[tt[act], bb[act], NO + NR + 2 + a_full[act]] = 1.0
    oh[:, :, VOC] = -C                                                   # scale row

    msk = (a_full[:, None, :] == np.arange(A)[None, :, None]).astype(np.float32)
    return oh, msk, C


def _bc_insert(ap, axis, count):
    """Insert a 0-stride (broadcast) dim of size `count` at position `axis`."""
    import concourse.bass as bass
    lst = [list(d) for d in ap.ap]
    lst.insert(axis, [0, count])
    return bass.AP(ap.tensor, ap.offset, lst)


def _build_nc(nsteps=TT, nreps=1):
    import concourse.bass as bass  # noqa: F401
    import concourse.bacc as bacc
    import concourse.mybir as mybir
    import concourse.tile as tile

    f32 = mybir.dt.float32
    f32r = mybir.dt.float32r
    bf16 = mybir.dt.bfloat16
    e4 = mybir.dt.float8e4
    e5 = mybir.dt.float8e5
    EXP = mybir.ActivationFunctionType.Exp
    CPY = mybir.ActivationFunctionType.Copy
    MUL = mybir.AluOpType.mult
    ADD = mybir.AluOpType.add
    DR = mybir.MatmulPerfMode.DoubleRow
    AX = mybir.AxisListType.X

    nc = bacc.Bacc(None, target_bir_lowering=False)

    oh_d = nc.dram_tensor("oh", [VOCC, TT * BC], f32r, kind="ExternalInput")
    tbl_d = nc.dram_tensor("tbl", [VOCC, S], f32r, kind="ExternalInput")
    pt_d = nc.dram_tensor("ptab", [128, A, KC, SP], e4, kind="ExternalInput")
    li_d = nc.dram_tensor("linit", [128, KC], f32, kind="ExternalInput")
    mk_d = nc.dram_tensor("msk", [128, TT, A, BC], bf16, kind="ExternalInput")
    id_d = nc.dram_tensor("ident", [BC, BC], bf16, kind="ExternalInput")
    out_d = nc.dram_tensor("out", [BC, TT], f32, kind="ExternalOutput")

    LOOK = 3                  # qm lookahead (steps)

    with tile.TileContext(nc) as tc, ExitStack() as ctx:
        const = ctx.enter_context(tc.tile_pool(name="const", bufs=1))
        qmpool = ctx.enter_context(tc.tile_pool(name="qm", bufs=LOOK + 3))
        w8pool = ctx.enter_context(tc.tile_pool(name="w8", bufs=2))
        spool = ctx.enter_context(tc.tile_pool(name="ssum", bufs=2))
        nrm = ctx.enter_context(tc.tile_pool(name="nrm", bufs=3))
        pp = ctx.enter_context(tc.tile_pool(name="ppsum", bufs=2, space="PSUM"))
        ptp = ctx.enter_context(tc.tile_pool(name="tpsum", bufs=1, space="PSUM"))
        lqp = pp

        ptab = const.tile([128, A, KC, SP], e4)
        qbuf = const.tile([128, KC, TT * BC], bf16)
        tbl = const.tile([VOCC, S], f32r)
        oh = const.tile([VOCC, TT * BC], f32r)
        ident = const.tile([BC, BC], bf16)
        alpha0 = const.tile([128, KC, 1], f32)
        li = const.tile([128, KC], f32)
        ellbuf = const.tile([BC, TT], f32)
        mall = const.tile([128, TT, A, BC], bf16)

        nc.sync.dma_start(tbl[:], tbl_d[:])
        nc.sync.dma_start(oh[:], oh_d[:])
        nc.sync.dma_start(ident[:], id_d[:])
        nc.sync.dma_start(li[:], li_d[:])
        nc.sync.dma_start(mall[:, 0:TT // 4], mk_d[:, 0:TT // 4])
        nc.sync.dma_start(ptab[:], pt_d[:])
        nc.scalar.activation(alpha0[:, :, 0], li[:], EXP)

        # emission rows: lq = tbl.T @ onehot (+C row), then q = exp(lq) -> bf16
        # j-outer so early time steps are ready first
        for j in range(NCOL):
            for mc in range(KC):
                lq = lqp.tile([128, NCHUNK], f32, tag="lq")
                nc.tensor.matmul(
                    lq[:],
                    tbl[:, mc * 128:(mc + 1) * 128],
                    oh[:, j * NCHUNK:(j + 1) * NCHUNK],
                    start=True, stop=True,
                )
                nc.scalar.activation(
                    qbuf[:, mc, j * NCHUNK:(j + 1) * NCHUNK], lq[:], EXP
                )

        # remaining mask chunks: deferred so they don't contend with the
        # startup-critical ptab/oh DMAs
        for ch in range(1, 4):
            t0c = (TT * ch) // 4
            t1c = (TT * (ch + 1)) // 4
            nc.sync.dma_start(mall[:, t0c:t1c], mk_d[:, t0c:t1c])

        def emit_qm(t):
            """qm[p,a,kc,b] = q_t[p,kc,b] * mask_t[p,a,b]  (bf16, off-chain)"""
            qm = qmpool.tile([128, A, KC, BC], bf16, tag="qm")
            qs = qbuf[:, :, t * BC:(t + 1) * BC]
            nc.vector.tensor_tensor(
                qm[:], _bc_insert(qs, 1, A), _bc_insert(mall[:, t], 2, KC), MUL
            )
            return qm

        def emit_ell(t, recip_prev, uh1_, want_recip=True):
            """s_t = (w8·R column) * recip_{t-1}; then recip_t = 1/s_t."""
            if recip_prev is None:
                nc.vector.tensor_copy(ellbuf[:, t:t + 1], uh1_[:, 256:257])
            else:
                nc.vector.tensor_tensor(ellbuf[:, t:t + 1], uh1_[:, 256:257],
                                        recip_prev[:], MUL)
            if not want_recip:
                return None
            rec = nrm.tile([BC, 1], f32, tag="rec")
            nc.vector.reciprocal(rec[:], ellbuf[:, t:t + 1])
            return rec

        for _rep in range(nreps):
         recip = None
         prev_ell = None          # (t, recip_prev, uh1) pending measurement
         qms = {t: emit_qm(t) for t in range(LOOK)}
         uT_prev = (None, None)
         for t in range(nsteps):
            qm = qms.pop(t)

            # w8[p,a,kc,b] = fp8e5(alpha[p,kc,b] * qm[p,a,kc,b]).  k-pair j0
            # is split into action-halves so the first matmul of the step
            # waits on a 258ns op, not a 392ns one.
            w8 = w8pool.tile([128, A, KC, BC], e5, tag="w8")
            def _w8src(ks, na):
                if t == 0 and _rep == 0:
                    return _bc_insert(_bc_insert(alpha0[:, ks, 0], 1, na), 3, BC)
                j = ks.start // 2
                return _bc_insert(uT_prev[j][:, :, :], 1, na)
            k0, k1 = slice(0, 2), slice(2, 4)
            for asl in (slice(0, 4), slice(4, 8)):
                nc.vector.tensor_tensor(w8[:, asl, k0, :], _w8src(k0, 4),
                                        qm[:, asl, k0, :], MUL)
            nc.vector.tensor_tensor(w8[:, :, k1, :], _w8src(k1, A),
                                    qm[:, :, k1, :], MUL)

            if t + LOOK < nsteps:
                qms[t + LOOK] = emit_qm(t + LOOK)

            # u = Σ_a w8_a @ P8[a] : DoubleRow matmuls split into N-halves
            # (separate PSUM tiles).  Phase order balances the two critical
            # cycles: j0L, j0R(a0-3), j1L, j0R(a4-7), j1R — so uh0 stops at
            # ~60% of the stream and the j1 operands (which trail the
            # previous step's second copy) are consumed late.  uh1 carries 4
            # extra columns: col 256 accumulates w8 · R (R = quantized-P8
            # row sums / 4) — the raw step mass lands in PSUM for free.
            uh0 = pp.tile([BC, S // 2], f32, tag="u0")
            uh1 = pp.tile([BC, S // 2 + 4], f32, tag="u1")
            cl, cr = slice(0, 256), slice(256, 516)
            phases = [(0, uh0, cl, range(A)), (0, uh1, cr, range(5)),
                      (1, uh0, cl, range(A)), (0, uh1, cr, range(5, A)),
                      (1, uh1, cr, range(A))]
            for j, u_, cols, arng in phases:
                for a in arng:
                    nc.tensor.matmul(
                        u_[:],
                        w8[:, a, 2 * j:2 * j + 2, :],
                        ptab[:, a, 2 * j:2 * j + 2, cols],
                        start=(j == 0 and a == 0),
                        stop=(j == 1 and a == A - 1),
                        perf_mode=DR,
                        skip_group_check=True,
                    )

            # c half 0 on DVE (lower output latency -> transposes start
            # sooner), half 1 on ACT (parallel engine); transpose each half
            # into its own uT tile.
            c = spool.tile([BC, S], bf16, tag="c")
            sc = recip[:] if recip is not None else 1.0
            last = t == nsteps - 1 and _rep == nreps - 1
            uTs = [None, None]
            for h, u_ in enumerate([uh0, uh1]):
                cols = slice(h * (S // 2), (h + 1) * (S // 2))
                if h == 0:
                    nc.vector.tensor_scalar(c[:, cols], u_[:, 0:256], sc,
                                            None, MUL)
                else:
                    nc.scalar.activation(c[:, cols], u_[:, 0:256], CPY,
                                         scale=sc)
                if not last:
                    uTh = ptp.tile([128, 2, BC], bf16, tag=f"uT{h}",
                                   name=f"uT{h}")
                    for k in range(2):
                        kc = 2 * h + k
                        nc.tensor.matmul(
                            uTh[:, k, :], c[:, kc * 128:(kc + 1) * 128],
                            ident[:], is_transpose=True,
                            start=True, stop=True,
                        )
                    uTs[h] = uTh
            uT_prev = tuple(uTs)

            recip = emit_ell(t, recip, uh1)

        nc.sync.dma_start(out_d[:], ellbuf[:])

    nc.compile()
    return nc


_NC = None


def _get_nc():
    global _NC
    if _NC is None:
        _NC = _build_nc()
    return _NC


def make_in_maps(regime, seq_o, seq_r, seq_d, seq_a,
                 log_init, log_trans, log_emit_o, log_emit_r, log_emit_d,
                 log_policy):
    import ml_dtypes

    oh, msk, C = _host_prep(
        np.asarray(regime), np.asarray(seq_o), np.asarray(seq_r),
        np.asarray(seq_d), np.asarray(seq_a),
        np.asarray(log_emit_o, np.float32), np.asarray(log_emit_r, np.float32),
        np.asarray(log_emit_d, np.float32), np.asarray(log_policy, np.float32),
    )
    tbl = np.concatenate(
        [log_emit_o, log_emit_r, log_emit_d, log_policy,
         np.ones((1, S), np.float32)], 0
    ).astype(np.float32)                                         # [91, 512]
    P8 = (512.0 * np.exp(np.asarray(log_trans, np.float64))).astype(np.float32)
    P8q = np.ascontiguousarray(
        P8.reshape(A, KC, 128, S).transpose(2, 0, 1, 3)
    ).astype(ml_dtypes.float8_e4m3)                              # [128,A,KC,S]
    ptab = np.zeros((128, A, KC, SP), ml_dtypes.float8_e4m3)
    ptab[:, :, :, :S] = P8q
    # col S: R/4 where R = row sums of the quantized P8 (mass measurement)
    ptab[:, :, :, S] = (P8q.astype(np.float32).sum(-1) / 4.0
                        ).astype(ml_dtypes.float8_e4m3)
    linit = np.ascontiguousarray(np.asarray(log_init, np.float32).reshape(KC, 128).T)
    ident = np.eye(BC, dtype=ml_dtypes.bfloat16)

    in_maps = []
    for c in range(NCORES):
        bs = c * BC
        ohc = np.ascontiguousarray(
            oh[:, bs:bs + BC, :].transpose(2, 0, 1).reshape(VOCC, TT * BC)
        )
        mskc = np.ascontiguousarray(
            np.broadcast_to(msk[None, :, :, bs:bs + BC], (128, TT, A, BC))
        ).astype(ml_dtypes.bfloat16)
        in_maps.append({
            "oh": ohc, "tbl": tbl, "ptab": ptab, "linit": linit,
            "msk": mskc, "ident": ident,
        })
    return in_maps, C


def kernel(regime, seq_o, seq_r, seq_d, seq_a,
           log_init, log_trans, log_emit_o, log_emit_r, log_emit_d,
           log_policy, _trace=False):
    from concourse.bass_utils import run_bass_kernel_spmd

    nc = _get_nc()
    in_maps, C = make_in_maps(
        regime, seq_o, seq_r, seq_d, seq_a, log_init, log_trans,
        log_emit_o, log_emit_r, log_emit_d, log_policy,
    )
    res = run_bass_kernel_spmd(nc, in_maps, core_ids=list(range(NCORES)),
                               trace=_trace)
    ell = np.concatenate([r["out"].reshape(BC, TT) for r in res.results])  # [B,TT]
    logp = (np.log(ell.astype(np.float64)).sum(1)
            + C.astype(np.float64).sum(0) - TT * LN512 + np.log(4.0))
    if _trace:
        kernel._last_results = res
    return logp.astype(np.float32)
